# revision 1
# baseline (speedup 1.0000x reference)
"""Trainium2 Bass kernel for DiffSelfAttention (B=1, T=2048, C=2048, 16 v-heads).

Sharding: tensor-parallel over heads across 8 NeuronCores. Core c owns
v-heads {2c, 2c+1} plus the matching q/k heads of both differential branches.
Each core computes its qkv slice, the attention for its 4 q/k heads, the
differential + per-head RMSNorm, and a partial projection
y_c = out_c @ w_proj[rows_c]. The host sums the 8 partials (unshard step).

Layout/strategy notes:
  - All matmuls run as float32r (full-rate fp32 on the PE at N>=256,
    ~2e-4 element rounding). DMA loads go directly into fp32r tiles;
    on-chip fp32r operands are produced by compute ops (engines round on
    write), which is what the BIR verifier requires.
  - q/k are produced directly transposed ([d, T]); v in natural layout
    ([T, d]); scores computed transposed ([tk, tq]) so probs@v needs no
    transposes anywhere.
  - Softmax divisions are eliminated: RMSNorm is invariant to any
    per-column positive scale, so instead of a1/r1 - lam*a2/r2 we feed it
    o' = a1*r2 - lam*a2*r1 (r = exp-sum broadcasts from a ones-matmul).
    The 1e-6 RMS eps is dropped: mean(o'^2) >> eps always for this data.
  - rsqrt for RMS is computed as exp(-0.5*log(m)) on the ACT engine
    (Reciprocal/Rsqrt activations are banned; Log+Exp share one ACT
    table set so there are no mid-kernel table switches).
  - Softmax column sums use two interleaved DVE accumulator chains so the
    serial dependency never gates the ACT exp stream.
"""

import math

import numpy as np

import concourse.bass as bass
import concourse.bacc as bacc
import concourse.mybir as mybir
import concourse.tile as tile

F32 = mybir.dt.float32
F32R = mybir.dt.float32r

T = 2048
C = 2048
N_HEAD = 16
H_DIM = 64
D2 = 2 * H_DIM  # 128 (v-head dim, also the RMS group size)
LAMBDA_INIT = 0.8 - 0.6 * math.exp(-0.3)
SCALE = 1.0 / math.sqrt(H_DIM)
P = 128
KSLABS = C // P  # 16 contraction slabs
TT = T // P  # 16 t-tiles
NCH = 512  # moving-operand chunk (max for 4-byte dtypes)
HQ = T // 2  # 1024-wide tq halves in the attention inner loop
N_CORES = 8

EXP = mybir.ActivationFunctionType.Exp
LOG = mybir.ActivationFunctionType.Ln
MULT = mybir.AluOpType.mult
ADD = mybir.AluOpType.add


def build(lam: float) -> bass.Bass:
    nc = bacc.Bacc("TRN2", target_bir_lowering=False, debug=False)

    xt_d = nc.dram_tensor("xt", [P, 4, KSLABS, NCH], F32R, kind="ExternalInput")
    wqk_d = nc.dram_tensor("wqk", [P, KSLABS, 4 * P], F32R, kind="ExternalInput")
    wv_d = nc.dram_tensor("wv", [P, KSLABS, 2 * D2], F32R, kind="ExternalInput")
    wp_d = nc.dram_tensor("wp", [P, 2, T], F32R, kind="ExternalInput")
    sv_d = nc.dram_tensor("sv", [P, 1], F32, kind="ExternalInput")
    y_d = nc.dram_tensor("y", [TT, P, T], F32, kind="ExternalOutput")

    with tile.TileContext(nc) as tc:
        with tc.tile_pool(name="persist", bufs=1) as persist:
            sv = persist.tile([P, 1], F32)
            ones_f = persist.tile([P, P], F32)
            ones = persist.tile([P, P], F32R)
            qk = persist.tile([P, 4, T], F32R)  # q1|q2|k1|k2, [d, T] layout
            vnat = persist.tile([P, TT, 2 * D2], F32R)  # v, [T, d] layout
            nc.sync.dma_start(out=sv, in_=sv_d[:])
            nc.vector.memset(ones_f, 1.0)
            nc.vector.tensor_copy(ones, ones_f)

            # ---------- phase 1: qkv projections ----------
            with tc.tile_pool(name="w1", bufs=1) as w1p, \
                 tc.tile_pool(name="xt", bufs=2) as xtp, \
                 tc.tile_pool(name="ps_qk", bufs=2, space="PSUM") as pqk, \
                 tc.tile_pool(name="ps_v", bufs=2, space="PSUM") as pvp:
                wqk = w1p.tile([P, KSLABS, 4 * P], F32R)
                wv = w1p.tile([P, KSLABS, 2 * D2], F32R)
                nc.sync.dma_start(out=wqk, in_=wqk_d[:])
                nc.sync.dma_start(out=wv, in_=wv_d[:])
                for n in range(T // NCH):  # 512-wide t chunks
                    xt = xtp.tile([P, KSLABS, NCH], F32R)
                    nc.sync.dma_start(out=xt, in_=xt_d[:, n, :, :])
                    for m in range(4):  # q1, q2, k1, k2
                        ps = pqk.tile([P, NCH], F32)
                        for k in range(KSLABS):
                            nc.tensor.matmul(
                                ps,
                                wqk[:, k, m * P:(m + 1) * P],
                                xt[:, k, :],
                                start=(k == 0),
                                stop=(k == KSLABS - 1),
                            )
                        nc.vector.tensor_copy(qk[:, m, n * NCH:(n + 1) * NCH], ps)
                    for t2 in range(NCH // P):  # t-tiles in this chunk
                        ps = pvp.tile([P, 2 * D2], F32)
                        for k in range(KSLABS):
                            nc.tensor.matmul(
                                ps,
                                xt[:, k, t2 * P:(t2 + 1) * P],
                                wv[:, k, :],
                                start=(k == 0),
                                stop=(k == KSLABS - 1),
                            )
                        nc.vector.tensor_copy(vnat[:, n * (NCH // P) + t2, :], ps)

            # ---------- phases 2+3 ----------
            with tc.tile_pool(name="wp", bufs=1) as wpp:
                wp = wpp.tile([P, 2, T], F32R)
                on = wpp.tile([P, 2, T], F32R)  # normed diff out, [d, T] per vh
                nc.sync.dma_start(out=wp, in_=wp_d[:])

                # ---------- phase 2: attention ----------
                # Both v-head streams (array rows 0-63 / 64-127) are packed
                # into shared [P, 2, NCH] tiles: one ACT exp covers both, and
                # the PE gets 6 matmuls per tk-slab (scores x2, pv x2,
                # colsum x2) so it never idles long enough for the HAM
                # clock-gate to re-throttle it to 1.2 GHz.
                with tc.tile_pool(name="ps_s", bufs=2, space="PSUM") as psp, \
                     tc.tile_pool(name="ps_a", bufs=1, space="PSUM") as pap, \
                     tc.tile_pool(name="ps_r", bufs=1, space="PSUM") as rp, \
                     tc.tile_pool(name="exp", bufs=4) as ep, \
                     tc.tile_pool(name="keep", bufs=1) as kp:
                    opk = kp.tile([P, 2, T], F32)  # scaled diff o', per vh
                    a1u = {}
                    r1l = {}
                    for br in range(2):
                        for q4 in range(4):  # 512-wide tq quarters
                            c0 = q4 * NCH
                            pa = pap.tile([P, 2, NCH], F32, tag="pa")
                            r = rp.tile([P, 2, NCH], F32, tag="r")
                            for k in range(TT):  # tk slabs
                                ps = psp.tile([P, 2, NCH], F32, tag="s")
                                et = ep.tile([P, 2, NCH], F32R, tag="er")
                                for vh in range(2):
                                    rows = slice(vh * H_DIM, (vh + 1) * H_DIM)
                                    nc.tensor.matmul(
                                        ps[:, vh, :],
                                        qk[rows, 2 + br, k * P:(k + 1) * P],
                                        qk[rows, br, c0:c0 + NCH],
                                        start=True,
                                        stop=True,
                                    )
                                nc.scalar.activation(et, ps, EXP, scale=SCALE)
                                for vh in range(2):
                                    nc.tensor.matmul(
                                        pa[:, vh, :],
                                        vnat[:, k, vh * D2:(vh + 1) * D2],
                                        et[:, vh, :],
                                        start=(k == 0),
                                        stop=(k == TT - 1),
                                    )
                                    nc.tensor.matmul(
                                        r[:, vh, :],
                                        ones,
                                        et[:, vh, :],
                                        start=(k == 0),
                                        stop=(k == TT - 1),
                                    )
                            if br == 0:
                                # keep unnormalized a1 and -lam*r1 for branch 2
                                a1u[q4] = kp.tile([P, 2, NCH], F32, tag=f"a1u{q4}", name=f"a1u{q4}")
                                nc.vector.tensor_copy(a1u[q4], pa)
                                r1l[q4] = kp.tile([P, 2, NCH], F32, tag=f"r1l{q4}", name=f"r1l{q4}")
                                nc.vector.tensor_scalar_mul(r1l[q4], r, -lam)
                            else:
                                # o' = a1*r2 - lam*a2*r1  (a per-column positive
                                # rescale of o; RMSNorm cancels it)
                                m1 = ep.tile([P, 2, NCH], F32, tag="m1")
                                nc.vector.tensor_mul(m1, a1u[q4], r)
                                m2 = ep.tile([P, 2, NCH], F32, tag="m2")
                                nc.vector.tensor_mul(m2, pa, r1l[q4])
                                nc.vector.tensor_add(opk[:, :, c0:c0 + NCH], m1, m2)
                    # RMS: rsqrt(mean o'^2) = exp(-0.5*ln(mean)). All Ln ops
                    # emitted before all Exp ops -> at most 2 ACT table loads.
                    psms = []
                    for vh in range(2):
                        for hf in range(2):
                            q0 = hf * HQ
                            sq = ep.tile([P, HQ], F32R, tag="er")
                            nc.vector.tensor_mul(sq, opk[:, vh, q0:q0 + HQ], opk[:, vh, q0:q0 + HQ])
                            psm = psp.tile([P, HQ], F32, tag="s")
                            for c2 in range(2):
                                nc.tensor.matmul(
                                    psm[:, c2 * NCH:(c2 + 1) * NCH],
                                    ones,
                                    sq[:, c2 * NCH:(c2 + 1) * NCH],
                                    start=True,
                                    stop=True,
                                )
                            ln = kp.tile([P, HQ], F32, tag=f"a1u{2 * vh + hf}", name=f"ln{vh}{hf}")
                            nc.scalar.activation(ln, psm, LOG, scale=1.0 / D2)
                            psms.append(ln)
                    for vh in range(2):
                        for hf in range(2):
                            q0 = hf * HQ
                            rsq = ep.tile([P, HQ], F32, tag="m1")
                            nc.scalar.activation(rsq, psms[2 * vh + hf], EXP, scale=-0.5)
                            nc.vector.scalar_tensor_tensor(
                                on[:, vh, q0:q0 + HQ],
                                opk[:, vh, q0:q0 + HQ],
                                sv, rsq, op0=MULT, op1=MULT,
                            )

                # ---------- phase 3: output projection (partial sum) ----------
                with tc.tile_pool(name="ps_y", bufs=4, space="PSUM") as pyp, \
                     tc.tile_pool(name="ysb", bufs=3) as yp:
                    for tt_i in range(TT):
                        ysb = yp.tile([P, T], F32)
                        for nch in range(T // NCH):
                            py = pyp.tile([P, NCH], F32)
                            for vh in range(2):
                                nc.tensor.matmul(
                                    py,
                                    on[:, vh, tt_i * P:(tt_i + 1) * P],
                                    wp[:, vh, nch * NCH:(nch + 1) * NCH],
                                    start=(vh == 0),
                                    stop=(vh == 1),
                                )
                            nc.vector.tensor_copy(ysb[:, nch * NCH:(nch + 1) * NCH], py)
                        nc.sync.dma_start(out=y_d[tt_i], in_=ysb)
    nc.finalize()
    return nc


def _core_inputs(x, w_qkv, w_proj, rms_scale):
    """Host-side shard prep: per-core weight slices + replicated x^T."""
    xt = np.ascontiguousarray(x.reshape(T, C).T)  # [C, T]
    xtr = np.ascontiguousarray(
        xt.reshape(KSLABS, P, T // NCH, NCH).transpose(1, 2, 0, 3)
    )
    sv = np.ascontiguousarray(
        (rms_scale.astype(np.float32) * np.float32(1.0 - LAMBDA_INIT)).reshape(P, 1)
    )
    maps = []
    for c in range(N_CORES):
        cols = [
            w_qkv[:, 0 * 1024 + c * P:0 * 1024 + (c + 1) * P],  # q1 heads 2c,2c+1
            w_qkv[:, 1 * 1024 + c * P:1 * 1024 + (c + 1) * P],  # q2
            w_qkv[:, 2 * 1024 + c * P:2 * 1024 + (c + 1) * P],  # k1
            w_qkv[:, 3 * 1024 + c * P:3 * 1024 + (c + 1) * P],  # k2
        ]
        wqk = np.concatenate(cols, axis=1)  # [C, 512]
        wqk = np.ascontiguousarray(wqk.reshape(KSLABS, P, 4 * P).transpose(1, 0, 2))
        wv = w_qkv[:, 2 * C + c * 2 * D2:2 * C + (c + 1) * 2 * D2]  # [C, 256]
        wv = np.ascontiguousarray(wv.reshape(KSLABS, P, 2 * D2).transpose(1, 0, 2))
        wp = w_proj[c * 2 * D2:(c + 1) * 2 * D2, :]  # [256, T]
        wp = np.ascontiguousarray(wp.reshape(2, P, T).transpose(1, 0, 2))
        maps.append({"xt": xtr, "wqk": wqk, "wv": wv, "wp": wp, "sv": sv})
    return maps


def kernel(x, w_qkv, w_proj, lambda_q1, lambda_k1, lambda_q2, lambda_k2, rms_scale):
    from concourse.bass_utils import run_bass_kernel_spmd

    x = np.asarray(x, dtype=np.float32)
    w_qkv = np.asarray(w_qkv, dtype=np.float32)
    w_proj = np.asarray(w_proj, dtype=np.float32)
    rms_scale = np.asarray(rms_scale, dtype=np.float32)
    lam1 = np.exp(np.sum(np.asarray(lambda_q1) * np.asarray(lambda_k1), dtype=np.float32))
    lam2 = np.exp(np.sum(np.asarray(lambda_q2) * np.asarray(lambda_k2), dtype=np.float32))
    lam = float(lam1 - lam2 + LAMBDA_INIT)

    nc = build(lam)
    in_maps = _core_inputs(x, w_qkv, w_proj, rms_scale)
    res = run_bass_kernel_spmd(nc, in_maps, core_ids=list(range(N_CORES)))
    y = np.zeros((TT, P, T), np.float32)
    for rmap in res.results:
        y += rmap["y"]
    return y.reshape(1, T, C)



# revision 3
# speedup vs baseline: 1.2912x; 1.2912x over previous
"""Trainium2 Bass kernel for DiffSelfAttention (B=1, T=2048, C=2048, 16 v-heads).

Sharding: tensor-parallel over heads across 8 NeuronCores. Core c owns
v-heads {2c, 2c+1} plus the matching q/k heads of both differential branches.
Each core computes its qkv slice, the attention for its 4 q/k heads, the
differential + per-head RMSNorm, and a partial projection
y_c = out_c @ w_proj[rows_c]. The host sums the 8 partials (unshard step).

Differences vs. the original fp32r version (477us):
  - All matmul operands are bf16 (PSUM accumulation stays fp32). This makes
    every LDWEIGHTS eligible for fast-weight-load, which was the hidden PE
    bottleneck in the attention slab loop (6 fp32 LDWs = 1164ns/slab > the
    1065ns of matmul streaming). Input DMA also halves.
  - The attention slab loop is software-pipelined with a one-slab rotation:
    iteration k emits exp(k) [ACT], scores(k+1) [PE], then pv/colsum(k-1)
    [PE]. pv(k-1) needs exp(k-1), which finished a full ACT period earlier,
    so the PE never head-of-line blocks on the ACT engine (the original
    emitted scores,exp,pv per slab and ping-ponged both engines at ~50%).
  - q4-outer / branch-inner block order; RMSNorm and the output projection
    run per 512-column q-chunk in the gap between blocks (using the freed
    pa/r PSUM banks), so phase 3 overlaps phase 2 instead of serializing.
  - Startup DMA is split into 4-slab groups interleaved with the weight
    groups so the first matmuls start ~2us in instead of ~35us.
  - One manual LoadActFuncSet of the natural_log_exp set before the first
    activation: Ln and Exp then coexist with zero mid-kernel table reloads
    (the auto-placement thrashes 7 loads between exp- and ln-only sets).
  - Softmax divisions eliminated as before: RMSNorm is invariant to any
    per-column positive scale, so o' = a1*r2 - lam*a2*r1 feeds it directly,
    and rsqrt is exp(-0.5*ln(m)) on ACT (Rsqrt activation is banned).
"""

import math

import numpy as np

import concourse.bass as bass
import concourse.bacc as bacc
import concourse.mybir as mybir
import concourse.tile as tile

F32 = mybir.dt.float32
BF16 = mybir.dt.bfloat16

T = 2048
C = 2048
N_HEAD = 16
H_DIM = 64
D2 = 2 * H_DIM  # 128 (v-head dim, also the RMS group size)
LAMBDA_INIT = 0.8 - 0.6 * math.exp(-0.3)
SCALE = 1.0 / math.sqrt(H_DIM)
P = 128
KSLABS = C // P  # 16 contraction slabs
TT = T // P  # 16 t-tiles
NCH = 512  # tq block width / psum bank width in fp32
N_CORES = 8

EXP = mybir.ActivationFunctionType.Exp
LOG = mybir.ActivationFunctionType.Ln
MULT = mybir.AluOpType.mult


def build(lam: float) -> bass.Bass:
    nc = bacc.Bacc("TRN2", target_bir_lowering=False, debug=False)

    xt_d = nc.dram_tensor("xt", [P, 4, KSLABS, NCH], BF16, kind="ExternalInput")
    wqk_d = nc.dram_tensor("wqk", [P, KSLABS, 4 * P], BF16, kind="ExternalInput")
    wv_d = nc.dram_tensor("wv", [P, KSLABS, 2 * D2], BF16, kind="ExternalInput")
    wp_d = nc.dram_tensor("wp", [P, 2, T], BF16, kind="ExternalInput")
    sv_d = nc.dram_tensor("sv", [P, 1], F32, kind="ExternalInput")
    y_d = nc.dram_tensor("y", [TT, P, T], F32, kind="ExternalOutput")

    # Pin the combined ln+exp activation table once, before any ACTIVATE.
    from concourse.hw_specs import get_activation_tables

    tabs = get_activation_tables(nc.m.arch)
    act_set_id = next(
        i for i, fns in enumerate(tabs.values()) if EXP in fns and LOG in fns
    )
    act_loaded = [False]

    def ensure_act_table():
        if not act_loaded[0]:
            act_loaded[0] = True
            nc.scalar.add_instruction(
                mybir.InstLoadActFuncSet(
                    name=nc.scalar.bass.get_next_instruction_name(),
                    act_func_set_id=act_set_id,
                )
            )

    with tile.TileContext(nc) as tc:
        with tc.tile_pool(name="persist", bufs=1) as persist:
            sv = persist.tile([P, 1], F32)
            ones = persist.tile([P, P], BF16)
            qk = persist.tile([P, 4, T], BF16)  # q1|q2|k1|k2, [d, T] layout
            vnat = persist.tile([P, TT, 2 * D2], BF16)  # v, [T, d] layout
            nc.sync.dma_start(out=sv, in_=sv_d[:])
            nc.vector.memset(ones, 1.0)

            # ---------- phase 1: qkv projections ----------
            with tc.tile_pool(name="w1", bufs=1) as w1p, \
                 tc.tile_pool(name="xt", bufs=2) as xtp, \
                 tc.tile_pool(name="ps_qk", bufs=2, space="PSUM") as pqk, \
                 tc.tile_pool(name="ps_v", bufs=2, space="PSUM") as pvp:
                wqk = w1p.tile([P, KSLABS, 4 * P], BF16)
                wv = w1p.tile([P, KSLABS, 2 * D2], BF16)
                for n in range(T // NCH):  # 512-wide t chunks
                    xt = xtp.tile([P, KSLABS, NCH], BF16)
                    if n == 0:
                        # ramp: interleave 4-slab groups of x^T with the
                        # matching weight groups so slab-0 matmuls start
                        # as soon as the first ~0.5MB lands
                        for g in range(0, KSLABS, 4):
                            nc.sync.dma_start(
                                out=xt[:, g:g + 4, :], in_=xt_d[:, 0, g:g + 4, :]
                            )
                            nc.sync.dma_start(
                                out=wqk[:, g:g + 4, :], in_=wqk_d[:, g:g + 4, :]
                            )
                        nc.sync.dma_start(out=wv, in_=wv_d[:])
                    else:
                        nc.sync.dma_start(out=xt, in_=xt_d[:, n, :, :])
                    for m in range(4):  # q1, q2, k1, k2
                        ps = pqk.tile([P, NCH], F32)
                        for k in range(KSLABS):
                            nc.tensor.matmul(
                                ps,
                                wqk[:, k, m * P:(m + 1) * P],
                                xt[:, k, :],
                                start=(k == 0),
                                stop=(k == KSLABS - 1),
                            )
                        nc.vector.tensor_copy(qk[:, m, n * NCH:(n + 1) * NCH], ps)
                    for t2 in range(NCH // P):  # t-tiles in this chunk
                        ps = pvp.tile([P, 2 * D2], F32)
                        for k in range(KSLABS):
                            nc.tensor.matmul(
                                ps,
                                xt[:, k, t2 * P:(t2 + 1) * P],
                                wv[:, k, :],
                                start=(k == 0),
                                stop=(k == KSLABS - 1),
                            )
                        nc.vector.tensor_copy(vnat[:, n * (NCH // P) + t2, :], ps)

            # ---------- phases 2+3 ----------
            with tc.tile_pool(name="wp", bufs=1) as wpp, \
                 tc.tile_pool(name="ps_s", bufs=2, space="PSUM") as psp, \
                 tc.tile_pool(name="ps_a", bufs=1, space="PSUM") as pap, \
                 tc.tile_pool(name="ps_r", bufs=1, space="PSUM") as rp, \
                 tc.tile_pool(name="exp", bufs=4) as ep, \
                 tc.tile_pool(name="keep", bufs=1) as kp, \
                 tc.tile_pool(name="ysb", bufs=2) as yp:
                wp = wpp.tile([P, 2, T], BF16)
                on = wpp.tile([P, 2, T], BF16)  # normed diff out, [d, T] per vh
                nc.sync.dma_start(out=wp, in_=wp_d[:])

                def scores(br, q4, k):
                    """scores^T for slab k into a fresh psum tile (both vh
                    packed as PE row-groups 0-63 / 64-127)."""
                    ps = psp.tile([P, 2, NCH], F32, tag="s")
                    c0 = q4 * NCH
                    for vh in range(2):
                        rows = slice(vh * H_DIM, (vh + 1) * H_DIM)
                        nc.tensor.matmul(
                            ps[:, vh, :],
                            qk[rows, 2 + br, k * P:(k + 1) * P],
                            qk[rows, br, c0:c0 + NCH],
                            start=True,
                            stop=True,
                        )
                    return ps

                def expo(ps):
                    ensure_act_table()
                    et = ep.tile([P, 2, NCH], BF16, tag="er")
                    nc.scalar.activation(et, ps, EXP, scale=SCALE)
                    return et

                def pv_cs(pa, r, et, k):
                    for vh in range(2):
                        nc.tensor.matmul(
                            pa[:, vh, :],
                            vnat[:, k, vh * D2:(vh + 1) * D2],
                            et[:, vh, :],
                            start=(k == 0),
                            stop=(k == TT - 1),
                        )
                        nc.tensor.matmul(
                            r[:, vh, :],
                            ones,
                            et[:, vh, :],
                            start=(k == 0),
                            stop=(k == TT - 1),
                        )

                def attn_block(br, q4, prolog):
                    """Rotated slab loop. `prolog` = (ps0, ps1, et0) emitted
                    earlier (before any boundary filler)."""
                    ps_prev, ps_cur, et_prev = prolog
                    pa = pap.tile([P, 2, NCH], F32, tag="pa")
                    r = rp.tile([P, 2, NCH], F32, tag="r")
                    for k in range(1, TT):
                        et_cur = expo(ps_cur)
                        if k + 1 < TT:
                            ps_nxt = scores(br, q4, k + 1)
                        pv_cs(pa, r, et_prev, k - 1)
                        ps_cur = ps_nxt if k + 1 < TT else None
                        et_prev = et_cur
                    pv_cs(pa, r, et_prev, TT - 1)
                    return pa, r

                def make_prolog(br, q4):
                    ps0 = scores(br, q4, 0)
                    ps1 = scores(br, q4, 1)
                    et0 = expo(ps0)
                    return (ps0, ps1, et0)

                def proj(q4):
                    """Partial projection + DMA-out for this q-chunk, using
                    the pa/r psum banks freed by the recombine."""
                    for tt2 in range(4):
                        ttg = q4 * 4 + tt2
                        ysb = yp.tile([P, T], F32)
                        for half in range(2):
                            pool = pap if half == 0 else rp
                            tag = "pa" if half == 0 else "r"
                            py = pool.tile([P, 2, NCH], F32, tag=tag)
                            for nch2 in range(2):
                                col0 = (half * 2 + nch2) * NCH
                                for vh in range(2):
                                    nc.tensor.matmul(
                                        py[:, nch2, :],
                                        on[:, vh, ttg * P:(ttg + 1) * P],
                                        wp[:, vh, col0:col0 + NCH],
                                        start=(vh == 0),
                                        stop=(vh == 1),
                                    )
                            nc.vector.tensor_copy(
                                ysb[:, half * 2 * NCH:(half + 1) * 2 * NCH], py
                            )
                        nc.sync.dma_start(out=y_d[ttg], in_=ysb)

                a1u = kp.tile([P, 2, NCH], F32, name="a1u")
                r1l = kp.tile([P, 2, NCH], F32, name="r1l")
                opk = kp.tile([P, 2, NCH], F32, name="opk")
                sq = kp.tile([P, 2, NCH], BF16, name="sq")
                lnt = kp.tile([P, 2, NCH], F32, name="lnt")
                rsq = kp.tile([P, 2, NCH], F32, name="rsq")

                prolog = make_prolog(0, 0)
                for q4 in range(4):
                    c0 = q4 * NCH
                    # ----- branch 1 (a1, r1) -----
                    pa, r = attn_block(0, q4, prolog)
                    prolog = make_prolog(1, q4)
                    # keep unnormalized a1 and -lam*r1 for branch 2
                    nc.vector.tensor_copy(a1u, pa)
                    nc.vector.tensor_scalar_mul(r1l, r, -lam)
                    # ----- branch 2 (a2, r2) + recombine -----
                    pa, r = attn_block(1, q4, prolog)
                    if q4 < 3:
                        prolog = make_prolog(0, q4 + 1)
                    # o' = a1*r2 - lam*a2*r1 (per-column positive rescale of
                    # o; RMSNorm cancels it)
                    nc.vector.tensor_mul(opk, a1u, r)
                    nc.vector.tensor_mul(a1u, pa, r1l)
                    nc.vector.tensor_add(opk, opk, a1u)
                    # ----- RMS for these 512 columns -----
                    nc.vector.tensor_mul(sq, opk, opk)
                    psm = psp.tile([P, 2, NCH], F32, tag="s")
                    for vh in range(2):
                        nc.tensor.matmul(
                            psm[:, vh, :], ones, sq[:, vh, :], start=True, stop=True
                        )
                    nc.scalar.activation(lnt, psm, LOG, scale=1.0 / D2)
                    nc.scalar.activation(rsq, lnt, EXP, scale=-0.5)
                    nc.vector.scalar_tensor_tensor(
                        on[:, :, c0:c0 + NCH], opk, sv, rsq, op0=MULT, op1=MULT
                    )
                    # ----- projection for these 512 rows of y -----
                    proj(q4)
    nc.finalize()
    return nc


def _core_inputs(x, w_qkv, w_proj, rms_scale):
    """Host-side shard prep: per-core weight slices + replicated x^T (bf16)."""
    import ml_dtypes

    bf16 = ml_dtypes.bfloat16
    xt = np.ascontiguousarray(x.reshape(T, C).T)  # [C, T]
    xtr = np.ascontiguousarray(
        xt.reshape(KSLABS, P, T // NCH, NCH).transpose(1, 2, 0, 3)
    ).astype(bf16)
    sv = np.ascontiguousarray(
        (rms_scale.astype(np.float32) * np.float32(1.0 - LAMBDA_INIT)).reshape(P, 1)
    )
    maps = []
    for c in range(N_CORES):
        cols = [
            w_qkv[:, 0 * 1024 + c * P:0 * 1024 + (c + 1) * P],  # q1 heads 2c,2c+1
            w_qkv[:, 1 * 1024 + c * P:1 * 1024 + (c + 1) * P],  # q2
            w_qkv[:, 2 * 1024 + c * P:2 * 1024 + (c + 1) * P],  # k1
            w_qkv[:, 3 * 1024 + c * P:3 * 1024 + (c + 1) * P],  # k2
        ]
        wqk = np.concatenate(cols, axis=1)  # [C, 512]
        wqk = np.ascontiguousarray(
            wqk.reshape(KSLABS, P, 4 * P).transpose(1, 0, 2)
        ).astype(bf16)
        wv = w_qkv[:, 2 * C + c * 2 * D2:2 * C + (c + 1) * 2 * D2]  # [C, 256]
        wv = np.ascontiguousarray(
            wv.reshape(KSLABS, P, 2 * D2).transpose(1, 0, 2)
        ).astype(bf16)
        wp = w_proj[c * 2 * D2:(c + 1) * 2 * D2, :]  # [256, T]
        wp = np.ascontiguousarray(wp.reshape(2, P, T).transpose(1, 0, 2)).astype(bf16)
        maps.append({"xt": xtr, "wqk": wqk, "wv": wv, "wp": wp, "sv": sv})
    return maps


def kernel(x, w_qkv, w_proj, lambda_q1, lambda_k1, lambda_q2, lambda_k2, rms_scale):
    from concourse.bass_utils import run_bass_kernel_spmd

    x = np.asarray(x, dtype=np.float32)
    w_qkv = np.asarray(w_qkv, dtype=np.float32)
    w_proj = np.asarray(w_proj, dtype=np.float32)
    rms_scale = np.asarray(rms_scale, dtype=np.float32)
    lam1 = np.exp(np.sum(np.asarray(lambda_q1) * np.asarray(lambda_k1), dtype=np.float32))
    lam2 = np.exp(np.sum(np.asarray(lambda_q2) * np.asarray(lambda_k2), dtype=np.float32))
    lam = float(lam1 - lam2 + LAMBDA_INIT)

    nc = build(lam)
    in_maps = _core_inputs(x, w_qkv, w_proj, rms_scale)
    res = run_bass_kernel_spmd(nc, in_maps, core_ids=list(range(N_CORES)))
    y = np.zeros((TT, P, T), np.float32)
    for rmap in res.results:
        y += rmap["y"]
    return y.reshape(1, T, C)


# revision 5
# speedup vs baseline: 1.5266x; 1.1823x over previous
"""Trainium2 Bass kernel for DiffSelfAttention (B=1, T=2048, C=2048, 16 v-heads).

Sharding: tensor-parallel over heads across 8 NeuronCores. Core c owns
v-heads {2c, 2c+1} plus the matching q/k heads of both differential branches.
Each core computes its qkv slice, the attention for its 4 q/k heads, the
differential + per-head RMSNorm, and a partial projection
y_c = out_c @ w_proj[rows_c]. The host sums the 8 partials (unshard step).

Performance structure (v3):
  - All matmul operands are bf16 (PSUM accumulation stays fp32): every
    LDWEIGHTS gets fast-weight-load (~98ns vs ~190ns fp32), which was the
    hidden PE bottleneck of the attention slab loop; input DMA halves.
  - Attention slab loop is software-pipelined with a 4-slab rotation:
    iteration k emits scores(k)+exp(k) and consumes pv/colsum(k-4), so the
    PE never head-of-line blocks on the ACT exp. The last 4 slabs' pv/cs
    ("leftovers") are emitted at the block boundary, after the next block's
    scores/exp prolog, keeping both engines fed across block transitions.
    Colsum leftovers go first so the recombine chain can start early.
  - PSUM is the hard constraint: scores 2x2 banks + pv accum 2 + colsum
    accum 2 = 8. The projection therefore runs as a final phase when all 8
    banks are free (quad-buffered), MM-paced, with PSUM->SBUF evacuation
    alternating between DVE and ACT. y partials are bf16 (halves the
    output DMA; host sums in fp32).
  - RMSNorm chain (recombine -> sq -> mean -> ln -> exp -> scale) runs on
    DVE/ACT in the shadow of the next block's slab loop; its two PE
    matmuls are spliced into that loop at iteration 8.
  - One manual LoadActFuncSet of the natural_log_exp set before the first
    activation: Ln and Exp coexist with zero mid-kernel table reloads.
  - Softmax divisions eliminated: RMSNorm is invariant to per-column
    positive scales, so o' = a1*r2 - lam*a2*r1 feeds it directly; rsqrt is
    exp(-0.5*ln(m)) (Rsqrt/Reciprocal activations are banned).
"""

import math

import numpy as np

import concourse.bass as bass
import concourse.bacc as bacc
import concourse.mybir as mybir
import concourse.tile as tile

F32 = mybir.dt.float32
BF16 = mybir.dt.bfloat16

T = 2048
C = 2048
N_HEAD = 16
H_DIM = 64
D2 = 2 * H_DIM  # 128 (v-head dim, also the RMS group size)
LAMBDA_INIT = 0.8 - 0.6 * math.exp(-0.3)
SCALE = 1.0 / math.sqrt(H_DIM)
P = 128
KSLABS = C // P  # 16 contraction slabs
TT = T // P  # 16 t-tiles
NCH = 512  # tq block width (one psum bank of fp32 per vh)
N_CORES = 8
PD = 4  # slab-loop rotation depth (prolog scores/exps, deferred pv/cs)

EXP = mybir.ActivationFunctionType.Exp
LOG = mybir.ActivationFunctionType.Ln
MULT = mybir.AluOpType.mult
ADD = mybir.AluOpType.add


def build(lam: float) -> bass.Bass:
    nc = bacc.Bacc("TRN2", target_bir_lowering=False, debug=False)

    xt_d = nc.dram_tensor("xt", [P, 4, KSLABS, NCH], BF16, kind="ExternalInput")
    wqk_d = nc.dram_tensor("wqk", [P, KSLABS, 4 * P], BF16, kind="ExternalInput")
    wv_d = nc.dram_tensor("wv", [P, KSLABS, 2 * D2], BF16, kind="ExternalInput")
    wp_d = nc.dram_tensor("wp", [P, 2, T], BF16, kind="ExternalInput")
    sv_d = nc.dram_tensor("sv", [P, 1], F32, kind="ExternalInput")
    y_d = nc.dram_tensor("y", [TT, P, T], BF16, kind="ExternalOutput")

    # Pin the combined ln+exp activation table once, before any ACTIVATE.
    from concourse.hw_specs import get_activation_tables

    tabs = get_activation_tables(nc.m.arch)
    act_set_id = next(
        i for i, fns in enumerate(tabs.values()) if EXP in fns and LOG in fns
    )
    act_loaded = [False]

    def ensure_act_table():
        if not act_loaded[0]:
            act_loaded[0] = True
            nc.scalar.add_instruction(
                mybir.InstLoadActFuncSet(
                    name=nc.scalar.bass.get_next_instruction_name(),
                    act_func_set_id=act_set_id,
                )
            )

    with tile.TileContext(nc) as tc:
        with tc.tile_pool(name="persist", bufs=1) as persist:
            sv = persist.tile([P, 1], F32)
            ones = persist.tile([P, P], BF16)
            qk = persist.tile([P, 4, T], BF16)  # q1|q2|k1|k2, [d, T] layout
            vnat = persist.tile([P, TT, 2 * D2], BF16)  # v, [T, d] layout
            nc.sync.dma_start(out=sv, in_=sv_d[:])
            nc.vector.memset(ones, 1.0)

            # ---------- phase 1: qkv projections ----------
            with tc.tile_pool(name="w1", bufs=1) as w1p, \
                 tc.tile_pool(name="xt", bufs=2) as xtp, \
                 tc.tile_pool(name="ps_qk", bufs=2, space="PSUM") as pqk, \
                 tc.tile_pool(name="ps_v", bufs=2, space="PSUM") as pvp:
                wqk = w1p.tile([P, KSLABS, 4 * P], BF16)
                wv = w1p.tile([P, KSLABS, 2 * D2], BF16)
                for n in range(T // NCH):  # 512-wide t chunks
                    xt = xtp.tile([P, KSLABS, NCH], BF16)
                    if n == 0:
                        # ramp: small first pieces on two dispatch queues
                        # (weights on the ACT hwdge, x^T on sync) so slab-0
                        # matmuls start as soon as ~0.5MB lands
                        nc.sync.dma_start(out=xt[:, 0:2, :], in_=xt_d[:, 0, 0:2, :])
                        nc.scalar.dma_start(out=wqk[:, 0:2, :], in_=wqk_d[:, 0:2, :])
                        nc.sync.dma_start(out=xt[:, 2:8, :], in_=xt_d[:, 0, 2:8, :])
                        nc.scalar.dma_start(out=wqk[:, 2:16, :], in_=wqk_d[:, 2:16, :])
                        nc.sync.dma_start(out=xt[:, 8:16, :], in_=xt_d[:, 0, 8:16, :])
                        nc.scalar.dma_start(out=wv, in_=wv_d[:])
                    else:
                        nc.sync.dma_start(out=xt, in_=xt_d[:, n, :, :])
                    for m in range(4):  # q1, q2, k1, k2
                        ps = pqk.tile([P, NCH], F32)
                        for k in range(KSLABS):
                            nc.tensor.matmul(
                                ps,
                                wqk[:, k, m * P:(m + 1) * P],
                                xt[:, k, :],
                                start=(k == 0),
                                stop=(k == KSLABS - 1),
                            )
                        nc.vector.tensor_copy(qk[:, m, n * NCH:(n + 1) * NCH], ps)
                    for t2 in range(NCH // P):  # t-tiles in this chunk
                        ps = pvp.tile([P, 2 * D2], F32)
                        for k in range(KSLABS):
                            nc.tensor.matmul(
                                ps,
                                xt[:, k, t2 * P:(t2 + 1) * P],
                                wv[:, k, :],
                                start=(k == 0),
                                stop=(k == KSLABS - 1),
                            )
                        nc.vector.tensor_copy(vnat[:, n * (NCH // P) + t2, :], ps)

            # ---------- phase 2: attention + RMS;  phase 3: projection ----------
            with tc.tile_pool(name="wp", bufs=1) as wpp, \
                 tc.tile_pool(name="ps_s", bufs=2, space="PSUM") as psp, \
                 tc.tile_pool(name="ps_a", bufs=1, space="PSUM") as pap, \
                 tc.tile_pool(name="ps_r", bufs=1, space="PSUM") as rp, \
                 tc.tile_pool(name="exp", bufs=10) as ep, \
                 tc.tile_pool(name="keep", bufs=1) as kp, \
                 tc.tile_pool(name="ysb", bufs=3) as yp:
                wp = wpp.tile([P, 2, T], BF16)
                on = wpp.tile([P, 2, T], BF16)  # normed diff out, [d, T] per vh
                nc.scalar.dma_start(out=wp, in_=wp_d[:])

                a1u = kp.tile([P, 2, NCH], F32, name="a1u")
                r1sb = kp.tile([P, 2, NCH], F32, name="r1sb")
                m1t = kp.tile([P, 2, NCH], F32, name="m1t")
                opk = kp.tile([P, 2, NCH], F32, name="opk")
                sq = kp.tile([P, 2, NCH], BF16, name="sq")
                lnt = kp.tile([P, 2, NCH], F32, name="lnt")
                rsqt = kp.tile([P, 2, NCH], F32, name="rsqt")

                class Blk:
                    def __init__(self, br, q4):
                        self.br, self.q4 = br, q4
                        self.ets = {}
                        self.pa = self.r = None

                def scores_pair(b, k):
                    ps = psp.tile([P, 2, NCH], F32, tag="s")
                    c0 = b.q4 * NCH
                    for vh in range(2):
                        rows = slice(vh * H_DIM, (vh + 1) * H_DIM)
                        nc.tensor.matmul(
                            ps[:, vh, :],
                            qk[rows, 2 + b.br, k * P:(k + 1) * P],
                            qk[rows, b.br, c0:c0 + NCH],
                            start=True,
                            stop=True,
                        )
                    return ps

                def expo(ps):
                    ensure_act_table()
                    et = ep.tile([P, 2, NCH], BF16, tag="er")
                    nc.scalar.activation(et, ps, EXP, scale=SCALE)
                    return et

                def pv_one(b, j, stop):
                    for vh in range(2):
                        nc.tensor.matmul(
                            b.pa[:, vh, :],
                            vnat[:, j, vh * D2:(vh + 1) * D2],
                            b.ets[j][:, vh, :],
                            start=(j == 0),
                            stop=stop,
                        )

                def cs_one(b, j, stop):
                    for vh in range(2):
                        nc.tensor.matmul(
                            b.r[:, vh, :],
                            ones,
                            b.ets[j][:, vh, :],
                            start=(j == 0),
                            stop=stop,
                        )

                def emit_prolog(b):
                    for j in range(PD):
                        b.ets[j] = expo(scores_pair(b, j))

                def emit_loop(b, fill=None):
                    b.pa = pap.tile([P, 2, NCH], F32, tag="pa")
                    b.r = rp.tile([P, 2, NCH], F32, tag="r")
                    for k in range(PD, TT):
                        b.ets[k] = expo(scores_pair(b, k))
                        pv_one(b, k - PD, stop=False)
                        cs_one(b, k - PD, stop=False)
                        if fill and k in fill:
                            for f in fill[k]:
                                f()

                def emit_leftovers(b):
                    # cs first: the recombine chain hangs off the r accum
                    for j in range(TT - PD, TT):
                        cs_one(b, j, stop=(j == TT - 1))
                    for j in range(TT - PD, TT):
                        pv_one(b, j, stop=(j == TT - 1))

                def make_rms_tail(q4):
                    def rms_tail():
                        psm = psp.tile([P, 2, NCH], F32, tag="s")
                        for vh in range(2):
                            nc.tensor.matmul(
                                psm[:, vh, :], ones, sq[:, vh, :],
                                start=True, stop=True,
                            )
                        nc.scalar.activation(lnt, psm, LOG, scale=1.0 / D2)
                        nc.scalar.activation(rsqt, lnt, EXP, scale=-0.5)
                        nc.vector.scalar_tensor_tensor(
                            on[:, :, q4 * NCH:(q4 + 1) * NCH],
                            opk, sv, rsqt, op0=MULT, op1=MULT,
                        )
                    return rms_tail

                cur = Blk(0, 0)
                emit_prolog(cur)
                emit_loop(cur)
                for q4 in range(4):
                    b0 = cur  # (br=0, q4): loop done, leftovers pending
                    b1 = Blk(1, q4)
                    emit_prolog(b1)
                    emit_leftovers(b0)
                    nc.vector.tensor_copy(a1u, b0.pa)  # unnormalized a1
                    nc.vector.tensor_copy(r1sb, b0.r)  # r1
                    emit_loop(b1)
                    if q4 < 3:
                        cur = Blk(0, q4 + 1)
                        emit_prolog(cur)
                    emit_leftovers(b1)
                    # o' = a1*r2 - lam*a2*r1 (per-column positive rescale of
                    # o; RMSNorm cancels it)
                    nc.vector.tensor_mul(m1t, a1u, b1.r)
                    nc.vector.tensor_mul(a1u, b1.pa, r1sb)
                    nc.vector.scalar_tensor_tensor(
                        opk, a1u, -lam, m1t, op0=MULT, op1=ADD
                    )
                    nc.vector.tensor_mul(sq, opk, opk)
                    if q4 < 3:
                        emit_loop(cur, fill={PD + 4: [make_rms_tail(q4)]})
                    else:
                        make_rms_tail(q4)()

                # ---------- phase 3: projection, all 8 psum banks ----------
                def py_tile(i):
                    if i % 4 == 0:
                        return pap.tile([P, 2, NCH], F32, tag="pa", name=f"py{i}")
                    if i % 4 == 1:
                        return rp.tile([P, 2, NCH], F32, tag="r", name=f"py{i}")
                    return psp.tile([P, 2, NCH], F32, tag="s", name=f"py{i}")

                pyi = 0
                for ttg in range(TT):
                    ysb = yp.tile([P, T], BF16)
                    for half in range(2):
                        py = py_tile(pyi)
                        pyi += 1
                        for nch2 in range(2):
                            col0 = (half * 2 + nch2) * NCH
                            for vh in range(2):
                                nc.tensor.matmul(
                                    py[:, nch2, :],
                                    on[:, vh, ttg * P:(ttg + 1) * P],
                                    wp[:, vh, col0:col0 + NCH],
                                    start=(vh == 0),
                                    stop=(vh == 1),
                                )
                        dst = ysb[:, half * 2 * NCH:(half + 1) * 2 * NCH]
                        if pyi % 2 == 0:
                            nc.vector.tensor_copy(dst, py)
                        else:
                            nc.scalar.copy(dst, py)
                    nc.sync.dma_start(out=y_d[ttg], in_=ysb)
    nc.finalize()
    return nc


def _core_inputs(x, w_qkv, w_proj, rms_scale):
    """Host-side shard prep: per-core weight slices + replicated x^T (bf16)."""
    import ml_dtypes

    bf16 = ml_dtypes.bfloat16
    xt = np.ascontiguousarray(x.reshape(T, C).T)  # [C, T]
    xtr = np.ascontiguousarray(
        xt.reshape(KSLABS, P, T // NCH, NCH).transpose(1, 2, 0, 3)
    ).astype(bf16)
    sv = np.ascontiguousarray(
        (rms_scale.astype(np.float32) * np.float32(1.0 - LAMBDA_INIT)).reshape(P, 1)
    )
    maps = []
    for c in range(N_CORES):
        cols = [
            w_qkv[:, 0 * 1024 + c * P:0 * 1024 + (c + 1) * P],  # q1 heads 2c,2c+1
            w_qkv[:, 1 * 1024 + c * P:1 * 1024 + (c + 1) * P],  # q2
            w_qkv[:, 2 * 1024 + c * P:2 * 1024 + (c + 1) * P],  # k1
            w_qkv[:, 3 * 1024 + c * P:3 * 1024 + (c + 1) * P],  # k2
        ]
        wqk = np.concatenate(cols, axis=1)  # [C, 512]
        wqk = np.ascontiguousarray(
            wqk.reshape(KSLABS, P, 4 * P).transpose(1, 0, 2)
        ).astype(bf16)
        wv = w_qkv[:, 2 * C + c * 2 * D2:2 * C + (c + 1) * 2 * D2]  # [C, 256]
        wv = np.ascontiguousarray(
            wv.reshape(KSLABS, P, 2 * D2).transpose(1, 0, 2)
        ).astype(bf16)
        wp = w_proj[c * 2 * D2:(c + 1) * 2 * D2, :]  # [256, T]
        wp = np.ascontiguousarray(wp.reshape(2, P, T).transpose(1, 0, 2)).astype(bf16)
        maps.append({"xt": xtr, "wqk": wqk, "wv": wv, "wp": wp, "sv": sv})
    return maps


def kernel(x, w_qkv, w_proj, lambda_q1, lambda_k1, lambda_q2, lambda_k2, rms_scale):
    from concourse.bass_utils import run_bass_kernel_spmd

    x = np.asarray(x, dtype=np.float32)
    w_qkv = np.asarray(w_qkv, dtype=np.float32)
    w_proj = np.asarray(w_proj, dtype=np.float32)
    rms_scale = np.asarray(rms_scale, dtype=np.float32)
    lam1 = np.exp(np.sum(np.asarray(lambda_q1) * np.asarray(lambda_k1), dtype=np.float32))
    lam2 = np.exp(np.sum(np.asarray(lambda_q2) * np.asarray(lambda_k2), dtype=np.float32))
    lam = float(lam1 - lam2 + LAMBDA_INIT)

    nc = build(lam)
    in_maps = _core_inputs(x, w_qkv, w_proj, rms_scale)
    res = run_bass_kernel_spmd(nc, in_maps, core_ids=list(range(N_CORES)))
    y = np.zeros((TT, P, T), np.float32)
    for rmap in res.results:
        y += np.asarray(rmap["y"], dtype=np.float32)
    return y.reshape(1, T, C)
